# revision 33
# baseline (speedup 1.0000x reference)
"""Trainium2 Bass kernel for the ConexaoRegional locally-connected layer.

Computation:  z[b, n, d, s] = sum_{h,w} region_n(x[b])[h, w] * pesos[n, d, s, h, w]
  x:     [32, 1, 256, 256] f32
  pesos: [4096, 16, 16, 4, 4] f32
  out:   [32, 4096, 16, 16] f32

Sharding: tensor-parallel over regions (N) across 8 cores. Core c handles
regions [512c, 512c+512) (region-rows [8c, 8c+8), x pixel rows [32c, 32c+32)).

Local region id mapping: n_local = 16*g + 4*i + hg with group g in [0,32),
row-group i in [0,4), col-group hg in [0,4). Groups are processed in pairs
gp = g//2, u = g%2.

Precision: the kernel is purely HBM-bandwidth-bound (f32 traffic = 26 MB/core
= 73us at the ~358 GB/s per-core roofline, which the f32 version hit). x and w
are fed as bf16 and the output is written as fp16 (PSUM accumulates in f32;
the PSUM->SBUF evacuation downcasts), for 13 MB/core ~= 37us roofline. Host
upcasts the fp16 output to f32. End-to-end rel err ~2e-3 (gate is 2e-2).

Matmul shape: W is the STATIONARY operand (lhsT) so its 128 ds-columns map to
PSUM partitions and the small batch dim is the moving stream. K=32 packs both
group parities u: x is zero-padded in the 16 K-rows of the other parity, so
the zero x rows nullify the other parity's W rows and W stays fully dense:
  wsb[32i + 16u + k, 1024gp + 256hg + 128e + d] = pesos_t[n, k, 128e + d]
    for n = 16(2gp+u) + 4i + hg
  xsb[32i + 16u' + k, 256gp + 64hg + 32u + b]
    = xr[b, n(2gp+u, i, hg), k] if u'==u else 0
512 matmuls/core: (gp, i, hg, e): psum[:, 512i+128hg+64e : +64] =
  wsb[32i:+32, wcol:+128]^T @ xsb[32i:+32, xcol:+64], tile_position (32i, 0).
PSUM banks: row-group i owns bank i of each psum buffer; same-position
matmuls (fixed i, varying hg/e) serialize safely on the PE sub-array,
cross-i matmuls never share a bank.

Engine/DMA orchestration (each dma_start blocks the issuing engine's SEQ for
~0.6-1.3us of HWDGE descgen, so DMA issue is kept off the copy engines'
critical path):
  - SP(sync): x load (1MB, issued first - nothing computes without x), then
    all 16 output stores (512KB each), each gated on that gp's copies.
  - DVE: W chunks q0,q2 issued up front, then per-gp PSUM copy cols [0:768).
  - Act: W chunks q1,q3 up front, then per-gp copy cols [768:1536).
  - Pool(gpsimd): per-gp copy cols [1536:2048) (Pool PSUM reads are slowest).
  - W chunk q (1MB) covers gps 4q..4q+3; matmul(gp) only waits on x + its
    chunk, so the PE starts after ~2 reads and stores interleave with the
    remaining W reads, keeping the DMA engines saturated end-to-end.
"""

import numpy as np

N_CORES = 8
B = 32
N_REG = 4096
DS = 256
K = 16
RPC = N_REG // N_CORES  # 512
NG = 32                 # groups per core
NP = NG // 2            # group pairs

_CACHE = {}


def _build_nc(reps=1, dyn_reps=1):
    import contextlib

    import concourse.bacc as bacc
    import concourse.mybir as mybir
    import concourse.tile as tile

    F32 = mybir.dt.float32
    BF16 = mybir.dt.bfloat16
    F16 = mybir.dt.float16
    nc = bacc.Bacc("TRN2", target_bir_lowering=False, debug=False)
    x_d = nc.dram_tensor("x", [128, 4096], BF16, kind="ExternalInput")
    w_d = nc.dram_tensor("w", [128, NP, 1024], BF16, kind="ExternalInput")
    o_d = nc.dram_tensor("o", [NP, 128, 2048], F16, kind="ExternalOutput")

    with tile.TileContext(nc) as tc:
        with (
            tc.tile_pool(name="xsb", bufs=2) as xp,
            tc.tile_pool(name="wsb", bufs=2) as wp,
            tc.tile_pool(name="ostage", bufs=6) as op,
            tc.tile_pool(name="pso", bufs=8, space="PSUM") as psop,
        ):
            wflat = w_d.ap().rearrange("p gp f -> p (gp f)")

            loop_cm = (
                tc.For_i(0, dyn_reps, 1)
                if dyn_reps > 1
                else contextlib.nullcontext()
            )
            with loop_cm:
                for rep in range(reps):
                    _one_rep(
                        nc, x_d, wflat, o_d, xp, wp, op, psop, BF16, F32, F16
                    )

    nc.compile()
    return nc


def _one_rep(nc, x_d, wflat, o_d, xp, wp, op, psop, BF16, F32, F16):
    # Per-rep tiles from bufs=2 pools: consecutive For_i iterations (with a
    # reps=2 unrolled body) double-buffer x/W so iteration i+1's loads
    # overlap iteration i's compute and stores.
    xsb = xp.tile([128, 4096], BF16)
    wsb = wp.tile([128, NP * 1024], BF16)
    nc.sync.dma_start(out=xsb[:], in_=x_d.ap())
    for q in range(4):
        nc.scalar.dma_start(
            out=wsb[:, 4096 * q : 4096 * (q + 1)],
            in_=wflat[:, 4096 * q : 4096 * (q + 1)],
        )
    # Copy-engine schedule for the 64 (gp, i) units. Only DVE and Act can
    # read PSUM (GPSIMD instructions cannot access PSUM on TRN2).
    copy_cycle = ["v", "a"]
    for gp in range(NP):
        ostage = op.tile([128, 2048], F16)
        for i in range(4):
            # One single-bank PSUM tile per (gp, i): 8 matmuls fill it, one
            # copy drains it. 8 units in flight keep every engine streaming.
            ps = psop.tile([128, 512], F32)
            for hg in range(4):
                xcol = 256 * gp + 64 * hg
                for e in range(2):
                    pcol = 128 * hg + 64 * e
                    wcol = 1024 * gp + 256 * hg + 128 * e
                    nc.tensor.matmul(
                        ps[:, pcol : pcol + 64],
                        wsb[32 * i : 32 * i + 32, wcol : wcol + 128],
                        xsb[32 * i : 32 * i + 32, xcol : xcol + 64],
                        start=True,
                        stop=True,
                        tile_position=(32 * i, 0),
                    )
            which = copy_cycle[(4 * gp + i) % 2]
            dst = ostage[:, 512 * i : 512 * (i + 1)]
            if which == "v":
                nc.vector.tensor_copy(out=dst, in_=ps[:])
            else:
                nc.scalar.copy(out=dst, in_=ps[:])
        nc.sync.dma_start(out=o_d.ap()[gp], in_=ostage[:])


def _prep_in_maps(x, pesos):
    """Full inputs -> list of 8 per-core input dicts (host-side layout prep)."""
    from ml_dtypes import bfloat16

    x = np.asarray(x, dtype=np.float32)
    pesos = np.asarray(pesos, dtype=np.float32)
    # pesos [n, d, s, h, w] -> [n, k=(h*4+w), ds=(d*16+s)]
    pesos_t = (
        np.ascontiguousarray(pesos.transpose(0, 3, 4, 1, 2))
        .reshape(N_REG, K, DS)
        .astype(bfloat16)
    )
    in_maps = []
    for c in range(N_CORES):
        # x regions for this core: [b, n_local, k]
        x_c = x[:, 0, 32 * c : 32 * c + 32, :]
        xr = (
            x_c.reshape(B, 8, 4, 64, 4)
            .transpose(0, 1, 3, 2, 4)
            .reshape(B, RPC, K)
            .astype(bfloat16)
        )
        # xt[32i + 16u' + k, 256gp + 64hg + 32u + b]
        #   = xr[b, 16(2gp+u)+4i+hg, k] if u'==u else 0
        arr = xr.reshape(B, NP, 2, 4, 4, K)      # b, gp, u, i, hg, k
        arr_t = arr.transpose(3, 5, 1, 4, 2, 0)  # i, k, gp, hg, u, b
        xt = np.zeros((4, 2, K, NP, 4, 2, B), dtype=bfloat16)
        xt[:, 0, :, :, :, 0] = arr_t[:, :, :, :, 0]
        xt[:, 1, :, :, :, 1] = arr_t[:, :, :, :, 1]
        xt = xt.reshape(128, 4096)

        # w[32i + 16u + k, gp, 256hg + ds]
        #   = pesos_t[512c + 16(2gp+u) + 4i + hg, k, ds]
        wc = pesos_t[512 * c : 512 * (c + 1)].reshape(NP, 2, 4, 4, K, DS)
        w_arr = np.ascontiguousarray(
            wc.transpose(2, 1, 4, 0, 3, 5)  # i, u, k, gp, hg, ds
        ).reshape(128, NP, 1024)

        in_maps.append({"x": np.ascontiguousarray(xt), "w": w_arr})
    return in_maps


def _unshard(results):
    """Per-core outputs -> full [B, N, 16, 16]."""
    out = np.empty((B, N_REG, DS), dtype=np.float32)
    for c, res in enumerate(results):
        o_c = np.asarray(res["o"], dtype=np.float32)
        # o[gp, p, 512i + 128hg + 64e + 32u + b] = out[b, n(2gp+u,i,hg), 128e+p]
        o_c = o_c.reshape(NP, 128, 4, 4, 2, 2, B)  # gp, p, i, hg, e, u, b
        o_t = o_c.transpose(6, 0, 5, 2, 3, 4, 1)   # b, gp, u, i, hg, e, p
        out[:, 512 * c : 512 * (c + 1), :] = o_t.reshape(B, RPC, DS)
    return out.reshape(B, N_REG, 16, 16)


def kernel(x, pesos):
    from concourse.bass_utils import run_bass_kernel_spmd

    if "nc" not in _CACHE:
        _CACHE["nc"] = _build_nc()
    nc = _CACHE["nc"]
    in_maps = _prep_in_maps(x, pesos)
    res = run_bass_kernel_spmd(nc, in_maps, core_ids=list(range(N_CORES)))
    return _unshard(res.results)


# revision 34
# speedup vs baseline: 1.1369x; 1.1369x over previous
"""Trainium2 Bass kernel for the ConexaoRegional locally-connected layer.

Computation:  z[b, n, d, s] = sum_{h,w} region_n(x[b])[h, w] * pesos[n, d, s, h, w]
  x:     [32, 1, 256, 256] f32
  pesos: [4096, 16, 16, 4, 4] f32
  out:   [32, 4096, 16, 16] f32

Sharding: tensor-parallel over regions (N) across 8 cores. Core c handles
regions [512c, 512c+512) (region-rows [8c, 8c+8), x pixel rows [32c, 32c+32)).

Local region id mapping: n_local = 16*g + 4*i + hg with group g in [0,32),
row-group i in [0,4), col-group hg in [0,4). Groups are processed in pairs
gp = g//2, u = g%2.

Precision: the kernel is purely HBM-bandwidth-bound (f32 traffic = 26 MB/core
= 73us at the ~358 GB/s per-core roofline, which the f32 version hit). x and w
are fed as bf16 and the output is written as fp16 (PSUM accumulates in f32;
the PSUM->SBUF evacuation downcasts), for 13 MB/core ~= 37us roofline. Host
upcasts the fp16 output to f32. End-to-end rel err ~2e-3 (gate is 2e-2).

Matmul shape: W is the STATIONARY operand (lhsT) so its 128 ds-columns map to
PSUM partitions and the small batch dim is the moving stream. K=32 packs both
group parities u: x is zero-padded in the 16 K-rows of the other parity, so
the zero x rows nullify the other parity's W rows and W stays fully dense:
  wsb[32i + 16u + k, 1024gp + 256hg + 128e + d] = pesos_t[n, k, 128e + d]
    for n = 16(2gp+u) + 4i + hg
  xsb[32i + 16u' + k, 256gp + 64hg + 32u + b]
    = xr[b, n(2gp+u, i, hg), k] if u'==u else 0
512 matmuls/core: (gp, i, hg, e): psum[:, 512i+128hg+64e : +64] =
  wsb[32i:+32, wcol:+128]^T @ xsb[32i:+32, xcol:+64], tile_position (32i, 0).
PSUM banks: row-group i owns bank i of each psum buffer; same-position
matmuls (fixed i, varying hg/e) serialize safely on the PE sub-array,
cross-i matmuls never share a bank.

Engine/DMA orchestration (each dma_start blocks the issuing engine's SEQ for
~0.6-1.3us of HWDGE descgen, so DMA issue is kept off the copy engines'
critical path):
  - SP(sync): x load (1MB, issued first - nothing computes without x), then
    all 16 output stores (512KB each), each gated on that gp's copies.
  - DVE: W chunks q0,q2 issued up front, then per-gp PSUM copy cols [0:768).
  - Act: W chunks q1,q3 up front, then per-gp copy cols [768:1536).
  - Pool(gpsimd): per-gp copy cols [1536:2048) (Pool PSUM reads are slowest).
  - W chunk q (1MB) covers gps 4q..4q+3; matmul(gp) only waits on x + its
    chunk, so the PE starts after ~2 reads and stores interleave with the
    remaining W reads, keeping the DMA engines saturated end-to-end.
"""

import numpy as np

N_CORES = 8
B = 32
N_REG = 4096
DS = 256
K = 16
RPC = N_REG // N_CORES  # 512
NG = 32                 # groups per core
NP = NG // 2            # group pairs

_CACHE = {}


def _build_nc(reps=1, dyn_reps=1):
    import contextlib

    import concourse.bacc as bacc
    import concourse.mybir as mybir
    import concourse.tile as tile

    F32 = mybir.dt.float32
    BF16 = mybir.dt.bfloat16
    F16 = mybir.dt.float16
    nc = bacc.Bacc("TRN2", target_bir_lowering=False, debug=False)
    x_d = nc.dram_tensor("x", [128, 4096], BF16, kind="ExternalInput")
    w_d = nc.dram_tensor("w", [128, NP, 1024], BF16, kind="ExternalInput")
    o_d = nc.dram_tensor("o", [NP, 128, 2048], F16, kind="ExternalOutput")

    with tile.TileContext(nc) as tc:
        with (
            tc.tile_pool(name="xsb", bufs=1) as xp,
            tc.tile_pool(name="wsb", bufs=1) as wp,
            tc.tile_pool(name="ostage", bufs=6) as op,
            tc.tile_pool(name="pso", bufs=8, space="PSUM") as psop,
        ):
            wflat = w_d.ap().rearrange("p gp f -> p (gp f)")

            loop_cm = (
                tc.For_i(0, dyn_reps, 1)
                if dyn_reps > 1
                else contextlib.nullcontext()
            )
            with loop_cm:
                for rep in range(reps):
                    _one_rep(
                        nc, x_d, wflat, o_d, xp, wp, op, psop, BF16, F32, F16
                    )

    nc.compile()
    return nc


def _one_rep(nc, x_d, wflat, o_d, xp, wp, op, psop, BF16, F32, F16):
    # Per-rep tiles from bufs=2 pools: consecutive For_i iterations (with a
    # reps=2 unrolled body) double-buffer x/W so iteration i+1's loads
    # overlap iteration i's compute and stores.
    xsb = xp.tile([128, 4096], BF16)
    wsb = wp.tile([128, NP * 1024], BF16)
    nc.sync.dma_start(out=xsb[:], in_=x_d.ap())
    for q in range(4):
        nc.scalar.dma_start(
            out=wsb[:, 4096 * q : 4096 * (q + 1)],
            in_=wflat[:, 4096 * q : 4096 * (q + 1)],
        )
    # Copy-engine schedule for the 64 (gp, i) units. Only DVE and Act can
    # read PSUM (GPSIMD instructions cannot access PSUM on TRN2).
    copy_cycle = ["v", "a"]
    for gp in range(NP):
        ostage = op.tile([128, 2048], F16)
        for i in range(4):
            # One single-bank PSUM tile per (gp, i): 8 matmuls fill it, one
            # copy drains it. 8 units in flight keep every engine streaming.
            ps = psop.tile([128, 512], F32)
            for hg in range(4):
                xcol = 256 * gp + 64 * hg
                for e in range(2):
                    pcol = 128 * hg + 64 * e
                    wcol = 1024 * gp + 256 * hg + 128 * e
                    nc.tensor.matmul(
                        ps[:, pcol : pcol + 64],
                        wsb[32 * i : 32 * i + 32, wcol : wcol + 128],
                        xsb[32 * i : 32 * i + 32, xcol : xcol + 64],
                        start=True,
                        stop=True,
                        tile_position=(32 * i, 0),
                    )
            which = copy_cycle[(4 * gp + i) % 2]
            dst = ostage[:, 512 * i : 512 * (i + 1)]
            if which == "v":
                nc.vector.tensor_copy(out=dst, in_=ps[:])
            else:
                nc.scalar.copy(out=dst, in_=ps[:])
        nc.sync.dma_start(out=o_d.ap()[gp], in_=ostage[:])


def _prep_in_maps(x, pesos):
    """Full inputs -> list of 8 per-core input dicts (host-side layout prep)."""
    from ml_dtypes import bfloat16

    x = np.asarray(x, dtype=np.float32)
    pesos = np.asarray(pesos, dtype=np.float32)
    # pesos [n, d, s, h, w] -> [n, k=(h*4+w), ds=(d*16+s)]
    pesos_t = (
        np.ascontiguousarray(pesos.transpose(0, 3, 4, 1, 2))
        .reshape(N_REG, K, DS)
        .astype(bfloat16)
    )
    in_maps = []
    for c in range(N_CORES):
        # x regions for this core: [b, n_local, k]
        x_c = x[:, 0, 32 * c : 32 * c + 32, :]
        xr = (
            x_c.reshape(B, 8, 4, 64, 4)
            .transpose(0, 1, 3, 2, 4)
            .reshape(B, RPC, K)
            .astype(bfloat16)
        )
        # xt[32i + 16u' + k, 256gp + 64hg + 32u + b]
        #   = xr[b, 16(2gp+u)+4i+hg, k] if u'==u else 0
        arr = xr.reshape(B, NP, 2, 4, 4, K)      # b, gp, u, i, hg, k
        arr_t = arr.transpose(3, 5, 1, 4, 2, 0)  # i, k, gp, hg, u, b
        xt = np.zeros((4, 2, K, NP, 4, 2, B), dtype=bfloat16)
        xt[:, 0, :, :, :, 0] = arr_t[:, :, :, :, 0]
        xt[:, 1, :, :, :, 1] = arr_t[:, :, :, :, 1]
        xt = xt.reshape(128, 4096)

        # w[32i + 16u + k, gp, 256hg + ds]
        #   = pesos_t[512c + 16(2gp+u) + 4i + hg, k, ds]
        wc = pesos_t[512 * c : 512 * (c + 1)].reshape(NP, 2, 4, 4, K, DS)
        w_arr = np.ascontiguousarray(
            wc.transpose(2, 1, 4, 0, 3, 5)  # i, u, k, gp, hg, ds
        ).reshape(128, NP, 1024)

        in_maps.append({"x": np.ascontiguousarray(xt), "w": w_arr})
    return in_maps


def _unshard(results):
    """Per-core outputs -> full [B, N, 16, 16]."""
    out = np.empty((B, N_REG, DS), dtype=np.float32)
    for c, res in enumerate(results):
        o_c = np.asarray(res["o"], dtype=np.float32)
        # o[gp, p, 512i + 128hg + 64e + 32u + b] = out[b, n(2gp+u,i,hg), 128e+p]
        o_c = o_c.reshape(NP, 128, 4, 4, 2, 2, B)  # gp, p, i, hg, e, u, b
        o_t = o_c.transpose(6, 0, 5, 2, 3, 4, 1)   # b, gp, u, i, hg, e, p
        out[:, 512 * c : 512 * (c + 1), :] = o_t.reshape(B, RPC, DS)
    return out.reshape(B, N_REG, 16, 16)


def kernel(x, pesos):
    from concourse.bass_utils import run_bass_kernel_spmd

    if "nc" not in _CACHE:
        _CACHE["nc"] = _build_nc()
    nc = _CACHE["nc"]
    in_maps = _prep_in_maps(x, pesos)
    res = run_bass_kernel_spmd(nc, in_maps, core_ids=list(range(N_CORES)))
    return _unshard(res.results)
